# revision 1
# baseline (speedup 1.0000x reference)
"""BatchAdaptiveConv2d Trainium2 kernel (8 NeuronCores, data parallel).

Math: out[b] = conv2d_same(x[b], W * wadapt[b, ci]) + bias * badapt[b]
 - wadapt[b] = cat(cond[b], lpe[b]) @ wa_w.T + wa_b    (per-sample Cin scale)
 - badapt[b] = cat(cond[b], lpe[b]) @ ba_w.T + ba_b    (per-sample Cout bias scale)

Per-core plan (2 samples each), high-K matmul formulation:
 - x tile [128 = (s, g, ci), 33, 258] bf16: g in {0,1} are row-shifted
   duplicates (at tile row t, g=0 holds x row t-1, g=1 holds x row t).
   g=1 is loaded once from HBM via gpsimd cast-DMA (f32 -> bf16 inline,
   so no on-chip conversion pass); g=0 is an on-chip SBUF->SBUF
   row-shifted duplicate plus a one-row HBM top-up. Cols 0/257 are the
   SAME zero pad.
 - Each PSUM block covers 8 output rows (2 stripes of 4 phases p):
   9 bf16 matmuls (3 row-groups k x 3 kw shifts) of K=64=(g,ci),
   M=128=(p,co), N=512=(2 stripes, 256 w). Row-group k reads tile row
   tl0+2k, supplying x rows {tl0+2k-1, tl0+2k}; tap (p,kh) lands in
   row-group k=(p+kh)//2, g=(p+kh)%2 -- exactly once each.
 - lhsT slot (3k+kw): [64=(g,ci), (p,co)] = W[ci,co,2k+g-p,kw]*wadapt[s,ci]
   built on-chip into a zeroed [128, 9, 4, 32] bf16 table via DVE copies.
 - PSUM -> SBUF via ScalarE Identity-activation with per-partition bias
   (= bias*badapt replicated x4 over phase groups), then wide f32 DMAs out
   on the scalar HWDGE ring (loads own the gpsimd SWDGE ring).
"""

import numpy as np

B, CIN, COUT, KK, H, W = 16, 32, 32, 3, 256, 256
EMB = 256
NCORES = 8
SB = B // NCORES  # samples per core

_CACHE = {}

RT = 32  # output rows per x tile (default; override via _build_nc(rt=...))
X_BF16 = False  # host-side bf16 pre-cast of x (measured slower on this HW)
WR = W + 2  # padded row width


def _build_nc(reps_loop=1, reps_mode="unroll", out_bf16=False, psum_bufs=8, xg_bufs=2, og_bufs=3, conv_chunks=3, xb_bufs=4, cast_dma=True, cast_out=False, rt=RT, sr_dup=True, x_bf16=X_BF16, skip_pe=False, skip_out=False, m64=True, dup_split=False):
    import concourse.bacc as bacc
    import concourse.bass as bass
    import concourse.mybir as mybir
    from concourse.tile import TileContext

    f32 = mybir.dt.float32
    bf16 = mybir.dt.bfloat16
    Identity = mybir.ActivationFunctionType.Identity

    nc = bacc.Bacc()

    x_dt = bf16 if x_bf16 else f32
    x_d = nc.declare_dram_parameter("x", [SB, CIN, H, W], x_dt, isOutput=False)
    cond_d = nc.declare_dram_parameter("condition", [SB, EMB], f32, isOutput=False)
    lpe_d = nc.declare_dram_parameter(
        "layer_pos_embedding", [SB, EMB], f32, isOutput=False
    )
    w_d = nc.declare_dram_parameter("weights", [CIN, COUT, KK, KK], f32, isOutput=False)
    bias_d = nc.declare_dram_parameter("bias", [COUT], f32, isOutput=False)
    waw_d = nc.declare_dram_parameter("wa_w", [CIN, 2 * EMB], f32, isOutput=False)
    wab_d = nc.declare_dram_parameter("wa_b", [CIN], f32, isOutput=False)
    baw_d = nc.declare_dram_parameter("ba_w", [COUT, 2 * EMB], f32, isOutput=False)
    bab_d = nc.declare_dram_parameter("ba_b", [COUT], f32, isOutput=False)
    out_dt = bf16 if out_bf16 else f32
    out_d = nc.declare_dram_parameter("out", [SB, COUT, H, W], out_dt, isOutput=True)

    def dram_ap(handle, offset, dims):
        a = handle[:]
        return bass.AP(tensor=a.tensor, offset=offset, ap=[list(d) for d in dims])

    RT = rt
    NT = H // RT
    HW_ = H * W  # 65536, per-channel plane
    SOFF = CIN * HW_  # per-sample x offset
    XP = (RT + 1) * WR  # per-partition x-tile elements

    with TileContext(nc) as tc:
        with (
            tc.tile_pool(name="const", bufs=1) as const,
            tc.tile_pool(name="xin", bufs=xg_bufs) as xpool,
            tc.tile_pool(name="xbp", bufs=xb_bufs) as xbpool,
            tc.tile_pool(name="ostg", bufs=og_bufs) as opool,
            tc.tile_pool(name="ps", bufs=psum_bufs, space="PSUM") as ppool,
        ):
            # ---- constants replicated x4 across partition quarter groups ----
            wrep = const.tile([128, COUT, KK, KK], f32, tag="wrep")
            nc.sync.dma_start(
                out=wrep.rearrange("q co kh kw -> q (co kh kw)"),
                in_=dram_ap(w_d, 0, [(0, 4), (288, 32), (1, 288)]),
            )
            waw_all = const.tile([128, 2 * EMB], f32, tag="waw_all")
            nc.sync.dma_start(
                out=waw_all, in_=dram_ap(waw_d, 0, [(0, 4), (512, 32), (1, 512)])
            )
            baw_all = const.tile([128, 2 * EMB], f32, tag="baw_all")
            nc.sync.dma_start(
                out=baw_all, in_=dram_ap(baw_d, 0, [(0, 4), (512, 32), (1, 512)])
            )
            wab_all = const.tile([128, 1], f32, tag="wab_all")
            nc.sync.dma_start(
                out=wab_all, in_=dram_ap(wab_d, 0, [(0, 4), (1, 32), (1, 1)])
            )
            bab_all = const.tile([128, 1], f32, tag="bab_all")
            nc.sync.dma_start(
                out=bab_all, in_=dram_ap(bab_d, 0, [(0, 4), (1, 32), (1, 1)])
            )
            bias_all = const.tile([128, 1], f32, tag="bias_all")
            nc.sync.dma_start(
                out=bias_all, in_=dram_ap(bias_d, 0, [(0, 4), (1, 32), (1, 1)])
            )

            # ib: [128=(s,g,ci), 512] = cat(cond[s], lpe[s]) per sample-half
            ib = const.tile([128, 2 * EMB], f32, tag="ib")
            for s in range(SB):
                nc.sync.dma_start(
                    out=ib[64 * s : 64 * s + 64, 0:EMB],
                    in_=dram_ap(cond_d, s * EMB, [(0, 64), (1, EMB)]),
                )
                nc.sync.dma_start(
                    out=ib[64 * s : 64 * s + 64, EMB : 2 * EMB],
                    in_=dram_ap(lpe_d, s * EMB, [(0, 64), (1, EMB)]),
                )

            # wadapt[(s,g,ci), 1] for both samples in one shot
            scr = const.tile([128, 2 * EMB], f32, tag="scr")
            wad = const.tile([128, 1], f32, tag="wad")
            nc.vector.tensor_mul(scr, waw_all, ib)
            nc.vector.reduce_sum(wad, scr, axis=mybir.AxisListType.X)
            nc.vector.tensor_add(wad, wad, wab_all)

            # modulated weights [(s,g,ci), co, kh, kw]
            wmod = const.tile([128, COUT, KK, KK], f32, tag="wmod")
            nc.vector.tensor_scalar_mul(
                wmod.rearrange("q co kh kw -> q (co kh kw)"),
                wrep.rearrange("q co kh kw -> q (co kh kw)"),
                wad,
            )

            # lhsT table [128=(s,g,ci), slot=3k+kw, p, co]:
            # slot holds wmod[.., co, 2k+g-p, kw] where 0<=2k+g-p<3, else 0
            lall = const.tile([128, 9, 4, 32], bf16, tag="lall")
            nc.vector.memset(lall.rearrange("q a b c -> q (a b c)"), 0.0)
            for s in range(SB):
                for g in range(2):
                    base = 64 * s + 32 * g
                    for k in range(3):
                        for p in range(4):
                            kh = 2 * k + g - p
                            if not (0 <= kh < KK):
                                continue
                            # dst [32, kw:3, 1, co:32] <- src transposed
                            nc.vector.tensor_copy(
                                lall[base : base + 32, 3 * k : 3 * k + 3, p : p + 1, :],
                                wmod[
                                    base : base + 32, :, kh : kh + 1, :
                                ].transpose([0, 3, 2, 1]),
                            )

            if m64:
                # lhsT table for the column-split scheme: 6 slots (k in {0,1}
                # x 3 kw) of [64=(g,ci), (p2,co)]; samples use disjoint 64x64
                # PE rectangles so their matmul streams overlap in the array.
                lall2 = const.tile([128, 6, 2, 32], bf16, tag="lall2")
                nc.vector.memset(lall2.rearrange("q a b c -> q (a b c)"), 0.0)
                for s in range(SB):
                    for g in range(2):
                        base = 64 * s + 32 * g
                        for k in range(2):
                            for p in range(2):
                                kh = 2 * k + g - p
                                if not (0 <= kh < KK):
                                    continue
                                nc.vector.tensor_copy(
                                    lall2[
                                        base : base + 32, 3 * k : 3 * k + 3, p : p + 1, :
                                    ],
                                    wmod[
                                        base : base + 32, :, kh : kh + 1, :
                                    ].transpose([0, 3, 2, 1]),
                                )
                # combined bias vec [ (s,g,co), 1 ] = bias[co]*badapt[s,co]
                # (ib already holds cat(cond[s],lpe[s]) on partition half s,
                #  and baw_all rows index co = q%32)
                scr3 = const.tile([128, 2 * EMB], f32, tag="scr3")
                badc = const.tile([128, 1], f32, tag="badc")
                nc.vector.tensor_mul(scr3, baw_all, ib)
                nc.vector.reduce_sum(badc, scr3, axis=mybir.AxisListType.X)
                nc.vector.tensor_add(badc, badc, bab_all)
                bvc = const.tile([128, 1], f32, tag="bvc")
                nc.vector.tensor_mul(bvc, badc, bias_all)

            # per-sample ACT bias vec [(x4, co), 1] = bias[co] * badapt[s, co]
            bvs = []
            for s in range(SB) if not m64 else ():
                ib2s = const.tile([128, 2 * EMB], f32, tag=f"ib2_{s}")
                nc.sync.dma_start(
                    out=ib2s[:, 0:EMB],
                    in_=dram_ap(cond_d, s * EMB, [(0, 128), (1, EMB)]),
                )
                nc.sync.dma_start(
                    out=ib2s[:, EMB : 2 * EMB],
                    in_=dram_ap(lpe_d, s * EMB, [(0, 128), (1, EMB)]),
                )
                scr2 = const.tile([128, 2 * EMB], f32, tag=f"scr2_{s}")
                bad = const.tile([128, 1], f32, tag=f"bad{s}")
                nc.vector.tensor_mul(scr2, baw_all, ib2s)
                nc.vector.reduce_sum(bad, scr2, axis=mybir.AxisListType.X)
                nc.vector.tensor_add(bad, bad, bab_all)
                bv = const.tile([128, 1], f32, tag=f"bv{s}")
                nc.vector.tensor_mul(bv, bad, bias_all)
                bvs.append(bv)

            # ---- main loop over row tiles ----
            def conv_body():
              for t in range(NT):
                  r0 = RT * t
                  if x_bf16:
                      # x pre-cast to bf16 on host; loads stay on the gpsimd
                      # ring so sync keeps the dup and scalar keeps the stores
                      xg = xbpool.tile([128, RT + 1, WR], bf16, tag="xb", name="xb")
                      ldma = nc.gpsimd.dma_start
                  elif cast_dma:
                      # gpsimd DMAs cast f32->bf16 inline: no raw f32 tile, no
                      # DVE conversion pass
                      xg = xbpool.tile([128, RT + 1, WR], bf16, tag="xb", name="xb")
                      ldma = nc.gpsimd.dma_start
                  else:
                      xg = xpool.tile([128, RT + 1, WR], f32, tag="xg", name="xg")
                      ldma = nc.sync.dma_start
                  # zero pad columns 0 and 257
                  nc.vector.memset(xg[:, :, 0:1], 0.0)
                  nc.vector.memset(xg[:, :, W + 1 : W + 2], 0.0)
                  # load: partition (s,g,ci) row tl holds x[s, ci, r0+tl+g-1]
                  if sr_dup:
                      # single HBM read: load only g=1 (x rows r0..r0+RT); the
                      # g=0 copy is an on-chip row-shifted duplicate, plus a
                      # one-row HBM top-up (x row r0-1) for tile row 0.
                      for s in range(SB):
                          hi = RT if t == NT - 1 else RT + 1
                          ldma(
                              out=xg[64 * s + 32 : 64 * s + 64, 0:hi, 1 : W + 1],
                              in_=dram_ap(
                                  x_d, s * SOFF + r0 * W, [(HW_, 32), (W, hi), (1, W)]
                              ),
                          )
                          if t == NT - 1:
                              nc.vector.memset(
                                  xg[64 * s + 32 : 64 * s + 64, RT : RT + 1, 1 : W + 1],
                                  0.0,
                              )
                          if t == 0:
                              nc.vector.memset(
                                  xg[64 * s : 64 * s + 32, 0:1, 1 : W + 1], 0.0
                              )
                          else:
                              ldma(
                                  out=xg[64 * s : 64 * s + 32, 0:1, 1 : W + 1],
                                  in_=dram_ap(
                                      x_d,
                                      s * SOFF + (r0 - 1) * W,
                                      [(HW_, 32), (W, 1), (1, W)],
                                  ),
                              )
                          dup_dma = nc.sync.dma_start
                          dup_bounds = (
                              (1, RT // 2 + 2, 0, RT // 2 + 1),
                              (RT // 2 + 2, RT + 1, RT // 2 + 1, RT),
                          ) if dup_split else ((1, RT + 1, 0, RT),)
                          for dlo, dhi, slo, shi in dup_bounds:
                              dup_dma(
                                  out=xg[64 * s : 64 * s + 32, dlo:dhi, 1 : W + 1],
                                  in_=xg[64 * s + 32 : 64 * s + 64, slo:shi, 1 : W + 1],
                              )
                  elif t == 0:
                      for lo, hi in ((0, 13), (13, RT + 1)):
                          for s in range(SB):
                              if lo == 0:
                                  nc.vector.memset(
                                      xg[64 * s : 64 * s + 32, 0:1, 1 : W + 1], 0.0
                                  )
                              glo = max(lo, 1)
                              ldma(
                                  out=xg[64 * s : 64 * s + 32, glo:hi, 1 : W + 1],
                                  in_=dram_ap(
                                      x_d,
                                      s * SOFF + (glo - 1) * W,
                                      [(HW_, 32), (W, hi - glo), (1, W)],
                                  ),
                              )
                              ldma(
                                  out=xg[64 * s + 32 : 64 * s + 64, lo:hi, 1 : W + 1],
                                  in_=dram_ap(
                                      x_d,
                                      s * SOFF + lo * W,
                                      [(HW_, 32), (W, hi - lo), (1, W)],
                                  ),
                              )
                  elif t == NT - 1:
                      for s in range(SB):
                          ldma(
                              out=xg[64 * s : 64 * s + 32, :, 1 : W + 1],
                              in_=dram_ap(
                                  x_d,
                                  s * SOFF + (r0 - 1) * W,
                                  [(HW_, 32), (W, RT + 1), (1, W)],
                              ),
                          )
                          ldma(
                              out=xg[64 * s + 32 : 64 * s + 64, 0:RT, 1 : W + 1],
                              in_=dram_ap(
                                  x_d, s * SOFF + r0 * W, [(HW_, 32), (W, RT), (1, W)]
                              ),
                          )
                          nc.vector.memset(
                              xg[64 * s + 32 : 64 * s + 64, RT : RT + 1, 1 : W + 1], 0.0
                          )
                  else:
                      for s in range(SB):
                          for g in range(2):
                              ldma(
                                  out=xg[
                                      64 * s + 32 * g : 64 * s + 32 * g + 32,
                                      :,
                                      1 : W + 1,
                                  ],
                                  in_=dram_ap(
                                      x_d,
                                      s * SOFF + (r0 + g - 1) * W,
                                      [(HW_, 32), (W, RT + 1), (1, W)],
                                  ),
                              )

                  if cast_dma or x_bf16:
                      xb = xg
                  else:
                      # f32 -> bf16 conversion pass (full 128 partitions)
                      xb = xbpool.tile([128, RT + 1, WR], bf16, tag="xb", name="xb")
                      nch = conv_chunks
                      bounds = [round(i * (RT + 1) / nch) for i in range(nch + 1)]
                      for lo, hi in zip(bounds[:-1], bounds[1:]):
                          nc.vector.tensor_copy(
                              xb[:, lo:hi, :].rearrange("q r w -> q (r w)"),
                              xg[:, lo:hi, :].rearrange("q r w -> q (r w)"),
                          )

                  # m64: samples on disjoint 64x64 PE rectangles (s0 rows
                  # 0-63 x cols 0-63, s1 rows 64-127 x cols 64-127) so their
                  # matmul streams overlap in the array. Blocks cover 4 output
                  # rows (2 stripes x 2 phases), 12 matmuls K=64 M=64 N=512.
                  for h2 in range(RT // 16) if m64 else ():
                      r2 = r0 + 16 * h2
                      ogc = opool.tile(
                          [128, 4, 2, 256], out_dt, tag="ogc", name="ogc"
                      )
                      for b2 in range(4):
                          tlb = 16 * h2 + 4 * b2
                          ps = ppool.tile([128, 512], f32, tag="ps")
                          for k in range(2):
                              row = tlb + 2 * k
                              for kw in range(3):
                                  for s in range(SB):
                                      nc.tensor.matmul(
                                          ps[64 * s : 64 * s + 64, :],
                                          lall2[
                                              64 * s : 64 * s + 64,
                                              3 * k + kw : 3 * k + kw + 1,
                                              :,
                                              :,
                                          ],
                                          bass.AP(
                                              tensor=xb.tensor,
                                              offset=xb[64 * s : 64 * s + 64].offset
                                              + row * WR
                                              + kw,
                                              ap=[[XP, 64], [2 * WR, 2], [1, W]],
                                          ),
                                          start=(k == 0 and kw == 0),
                                          stop=(k == 1 and kw == 2),
                                          skip_group_check=True,
                                      )
                          nc.scalar.activation(
                              ogc[:, b2 : b2 + 1, :, :],
                              ps[:],
                              Identity,
                              bias=bvc,
                              scale=1.0,
                          )
                      for s in range(SB):
                          for p in range(2):
                              nc.scalar.dma_start(
                                  out=dram_ap(
                                      out_d,
                                      s * SOFF + (r2 + p) * W,
                                      [(HW_, 32), (4 * W, 4), (2 * W, 2), (1, W)],
                                  ),
                                  in_=ogc[
                                      64 * s + 32 * p : 64 * s + 32 * p + 32, :, :, :
                                  ],
                              )

                  # blocks: 8 output rows each (2 stripes x 4 phases)
                  for h2 in range(RT // 32) if not m64 else ():
                   r2 = r0 + 32 * h2
                   ogs = [
                      opool.tile([128, 4, 2, 256], f32 if cast_out else out_dt, tag=f"og{s}", name=f"og{s}")
                      for s in range(SB)
                   ]
                   if skip_pe and not skip_out:
                       for s in range(SB):
                           nc.vector.memset(
                               ogs[s].rearrange("q a b c -> q (a b c)"), 0.0
                           )
                   for b2 in range(4) if not skip_pe else ():  # psum blocks
                      tl0 = 32 * h2 + 8 * b2
                      for s in range(SB):
                          ps = ppool.tile([128, 512], f32, tag="ps")
                          for k in range(3):
                              row = tl0 + 2 * k
                              for kw in range(3):
                                  lhsT = lall[
                                      64 * s : 64 * s + 64,
                                      3 * k + kw : 3 * k + kw + 1,
                                      :,
                                      :,
                                  ]
                                  rhs = bass.AP(
                                      tensor=xb.tensor,
                                      offset=xb[64 * s : 64 * s + 64].offset
                                      + row * WR
                                      + kw,
                                      ap=[[XP, 64], [4 * WR, 2], [1, W]],
                                  )
                                  nc.tensor.matmul(
                                      ps[:],
                                      lhsT,
                                      rhs,
                                      start=(k == 0 and kw == 0),
                                      stop=(k == 2 and kw == 2),
                                  )
                          nc.scalar.activation(
                              ogs[s][:, b2 : b2 + 1, :, :],
                              ps[:],
                              Identity,
                              bias=bvs[s],
                              scale=1.0,
                          )
                  # store 32 rows per sample: 4 DMAs (one per phase p)
                   odma = nc.gpsimd.dma_start if cast_out else nc.scalar.dma_start
                   for s in range(SB) if not skip_out else ():
                      for p in range(4):
                          odma(
                              out=dram_ap(
                                  out_d,
                                  s * SOFF + (r2 + p) * W,
                                  [(HW_, 32), (8 * W, 4), (4 * W, 2), (1, W)],
                              ),
                              in_=ogs[s][32 * p : 32 * p + 32, :, :, :],
                          )

            if reps_mode == "unroll" and reps_loop > 1:
                for _ in range(reps_loop):
                    conv_body()
            elif reps_loop > 1:
                with tc.For_i(0, reps_loop, 1):
                    conv_body()
            else:
                conv_body()

    nc.finalize()
    return nc


def _get_nc():
    if "nc" not in _CACHE:
        _CACHE["nc"] = _build_nc()
    return _CACHE["nc"]


def kernel(**inputs):
    from concourse.bass_utils import run_bass_kernel_spmd

    nc = _get_nc()
    res = _run(nc, inputs, run_bass_kernel_spmd, trace=False)
    return _gather(res)


def _x_host(x):
    # match the device kernel's declared x dtype
    if not X_BF16:
        return np.ascontiguousarray(x, dtype=np.float32)
    import ml_dtypes

    return np.ascontiguousarray(x.astype(ml_dtypes.bfloat16))


def _run(nc, inputs, run_bass_kernel_spmd, trace=False, trace_kwargs=None):
    in_maps = []
    for c in range(NCORES):
        s = slice(c * SB, (c + 1) * SB)
        in_maps.append(
            {
                "x": _x_host(inputs["x"][s]),
                "condition": np.ascontiguousarray(
                    inputs["condition"][s], dtype=np.float32
                ),
                "layer_pos_embedding": np.ascontiguousarray(
                    inputs["layer_pos_embedding"][s], dtype=np.float32
                ),
                "weights": np.ascontiguousarray(inputs["weights"], dtype=np.float32),
                "bias": np.ascontiguousarray(inputs["bias"], dtype=np.float32),
                "wa_w": np.ascontiguousarray(inputs["wa_w"], dtype=np.float32),
                "wa_b": np.ascontiguousarray(inputs["wa_b"], dtype=np.float32),
                "ba_w": np.ascontiguousarray(inputs["ba_w"], dtype=np.float32),
                "ba_b": np.ascontiguousarray(inputs["ba_b"], dtype=np.float32),
            }
        )
    kwargs = {}
    if trace:
        kwargs["trace"] = True
        if trace_kwargs:
            kwargs["trace_kwargs"] = trace_kwargs
    return run_bass_kernel_spmd(nc, in_maps, core_ids=list(range(NCORES)), **kwargs)


def _gather(res):
    return np.concatenate(
        [res.results[c]["out"] for c in range(NCORES)], axis=0
    ).astype(np.float32)



# revision 13
# speedup vs baseline: 2.6457x; 2.6457x over previous
"""BatchAdaptiveConv2d Trainium2 kernel (8 NeuronCores, data parallel).

Math: out[b] = conv2d_same(x[b], W * wadapt[b, ci]) + bias * badapt[b]
 - wadapt[b] = cat(cond[b], lpe[b]) @ wa_w.T + wa_b    (per-sample Cin scale)
 - badapt[b] = cat(cond[b], lpe[b]) @ ba_w.T + ba_b    (per-sample Cout bias scale)

Per-core plan (2 samples per core), m64 high-K matmul formulation:
 - x is zero-padded + bf16-cast on the HOST to [SB, CIN, 258, 258]: device
   loads are fully contiguous (no memsets / edge cases) and issued in
   ~4-row chunks so DMA descriptors stay in the ~2KB sweet spot.
 - x tile [128 = (s, g, ci), RT+1, 258] bf16: g=0 holds x row t-1, g=1
   holds x row t at tile row t. g=0 loaded from HBM; g=1 is an on-chip
   row-shifted SBUF->SBUF copy (chunked) + a 1-row HBM top-up.
 - Each PSUM block covers 4 output rows (2 stripes x 2 phases p); the two
   samples run on disjoint 64x64 PE rectangles so their matmul streams
   overlap: 12 bf16 matmuls K=64=(g,ci), M=64=(p,co), N=512 per block.
 - lhsT slots [64=(g,ci), (p,co)] = W[ci,co,2k+g-p,kw]*wadapt[s,ci] built
   on-chip into a zeroed [128, 6, 2, 32] bf16 table via DVE copies.
 - PSUM -> SBUF via ScalarE Identity-activation with per-partition bias
   (bias*badapt at (s,p?,co)), writing bf16.
 - Output dram layout is PHASE-PLANES [SB, 2, COUT, H/2, W] bf16 so each
   partition's store is 8 consecutive plane rows = 4KB contiguous runs;
   the host de-interleaves phases and casts to f32 in _gather.
"""

import numpy as np

B, CIN, COUT, KK, H, W = 16, 32, 32, 3, 256, 256
EMB = 256
NCORES = 8
SB = B // NCORES  # samples per core
PH, PW = H + 2, W + 2  # host-padded x plane (258 x 258)
OH = H // 2  # phase-plane rows

_CACHE = {}

RT = 32  # output rows per x tile


def _build_nc(reps_loop=1, reps_mode="unroll", out_bf16=True, rt=RT,
              psum_bufs=8, x_bufs=3, og_bufs=3, ldchunk=4):
    import concourse.bacc as bacc
    import concourse.bass as bass
    import concourse.mybir as mybir
    from concourse.tile import TileContext

    f32 = mybir.dt.float32
    bf16 = mybir.dt.bfloat16
    Identity = mybir.ActivationFunctionType.Identity

    nc = bacc.Bacc()

    x_d = nc.declare_dram_parameter("x", [SB, CIN, PH, PW], bf16, isOutput=False)
    cond_d = nc.declare_dram_parameter("condition", [SB, EMB], f32, isOutput=False)
    lpe_d = nc.declare_dram_parameter(
        "layer_pos_embedding", [SB, EMB], f32, isOutput=False
    )
    w_d = nc.declare_dram_parameter("weights", [CIN, COUT, KK, KK], f32, isOutput=False)
    bias_d = nc.declare_dram_parameter("bias", [COUT], f32, isOutput=False)
    waw_d = nc.declare_dram_parameter("wa_w", [CIN, 2 * EMB], f32, isOutput=False)
    wab_d = nc.declare_dram_parameter("wa_b", [CIN], f32, isOutput=False)
    baw_d = nc.declare_dram_parameter("ba_w", [COUT, 2 * EMB], f32, isOutput=False)
    bab_d = nc.declare_dram_parameter("ba_b", [COUT], f32, isOutput=False)
    out_dt = bf16 if out_bf16 else f32
    out_d = nc.declare_dram_parameter(
        "out", [SB, 2, COUT, OH, W], out_dt, isOutput=True
    )

    def dram_ap(handle, offset, dims):
        a = handle[:]
        return bass.AP(tensor=a.tensor, offset=offset, ap=[list(d) for d in dims])

    RT = rt
    NT = H // RT
    HW_ = H * W
    PP = PH * PW  # per-channel padded x plane elems
    SPX = CIN * PP  # per-sample padded x elems
    XP = (RT + 2) * PW  # per-partition x-tile elements
    OPLANE = COUT * OH * W  # one (s, p) output plane

    with TileContext(nc) as tc:
        with (
            tc.tile_pool(name="const", bufs=1) as const,
            tc.tile_pool(name="xin", bufs=x_bufs) as xpool,
            tc.tile_pool(name="ostg", bufs=og_bufs) as opool,
            tc.tile_pool(name="ps", bufs=psum_bufs, space="PSUM") as ppool,
        ):
            # ---- constants replicated x4 across partition quarter groups ----
            wrep = const.tile([128, COUT, KK, KK], f32, tag="wrep")
            nc.sync.dma_start(
                out=wrep.rearrange("q co kh kw -> q (co kh kw)"),
                in_=dram_ap(w_d, 0, [(0, 4), (288, 32), (1, 288)]),
            )
            waw_all = const.tile([128, 2 * EMB], f32, tag="waw_all")
            nc.sync.dma_start(
                out=waw_all, in_=dram_ap(waw_d, 0, [(0, 4), (512, 32), (1, 512)])
            )
            baw_all = const.tile([128, 2 * EMB], f32, tag="baw_all")
            nc.sync.dma_start(
                out=baw_all, in_=dram_ap(baw_d, 0, [(0, 4), (512, 32), (1, 512)])
            )
            wab_all = const.tile([128, 1], f32, tag="wab_all")
            nc.sync.dma_start(
                out=wab_all, in_=dram_ap(wab_d, 0, [(0, 4), (1, 32), (1, 1)])
            )
            bab_all = const.tile([128, 1], f32, tag="bab_all")
            nc.sync.dma_start(
                out=bab_all, in_=dram_ap(bab_d, 0, [(0, 4), (1, 32), (1, 1)])
            )
            bias_all = const.tile([128, 1], f32, tag="bias_all")
            nc.sync.dma_start(
                out=bias_all, in_=dram_ap(bias_d, 0, [(0, 4), (1, 32), (1, 1)])
            )

            # ib: [128=(s,g,ci), 512] = cat(cond[s], lpe[s]) per sample-half
            ib = const.tile([128, 2 * EMB], f32, tag="ib")
            for s in range(SB):
                nc.sync.dma_start(
                    out=ib[64 * s : 64 * s + 64, 0:EMB],
                    in_=dram_ap(cond_d, s * EMB, [(0, 64), (1, EMB)]),
                )
                nc.sync.dma_start(
                    out=ib[64 * s : 64 * s + 64, EMB : 2 * EMB],
                    in_=dram_ap(lpe_d, s * EMB, [(0, 64), (1, EMB)]),
                )

            # wadapt[(s,g,ci), 1] for both samples in one shot
            scr = const.tile([128, 2 * EMB], f32, tag="scr")
            wad = const.tile([128, 1], f32, tag="wad")
            nc.vector.tensor_mul(scr, waw_all, ib)
            nc.vector.reduce_sum(wad, scr, axis=mybir.AxisListType.X)
            nc.vector.tensor_add(wad, wad, wab_all)

            # modulated weights [(s,g,ci), co, kh, kw]
            wmod = const.tile([128, COUT, KK, KK], f32, tag="wmod")
            nc.vector.tensor_scalar_mul(
                wmod.rearrange("q co kh kw -> q (co kh kw)"),
                wrep.rearrange("q co kh kw -> q (co kh kw)"),
                wad,
            )

            # lhsT table: 6 slots (k in {0,1} x 3 kw) of [64=(g,ci), (p2,co)];
            # samples use disjoint 64x64 PE rectangles.
            lall2 = const.tile([128, 6, 2, 32], bf16, tag="lall2")
            nc.vector.memset(lall2.rearrange("q a b c -> q (a b c)"), 0.0)
            for s in range(SB):
                for g in range(2):
                    base = 64 * s + 32 * g
                    for k in range(2):
                        for p in range(2):
                            kh = 2 * k + g - p
                            if not (0 <= kh < KK):
                                continue
                            nc.vector.tensor_copy(
                                lall2[
                                    base : base + 32, 3 * k : 3 * k + 3, p : p + 1, :
                                ],
                                wmod[
                                    base : base + 32, :, kh : kh + 1, :
                                ].transpose([0, 3, 2, 1]),
                            )
            # combined bias vec [(s,g->p,co), 1] = bias[co]*badapt[s,co]
            scr3 = const.tile([128, 2 * EMB], f32, tag="scr3")
            badc = const.tile([128, 1], f32, tag="badc")
            nc.vector.tensor_mul(scr3, baw_all, ib)
            nc.vector.reduce_sum(badc, scr3, axis=mybir.AxisListType.X)
            nc.vector.tensor_add(badc, badc, bab_all)
            bvc = const.tile([128, 1], f32, tag="bvc")
            nc.vector.tensor_mul(bvc, badc, bias_all)

            # ---- main loop over row tiles ----
            def conv_body():
                for t in range(NT):
                    r0 = RT * t
                    # tile rows: g=0 holds padded rows r0+u (u in [0,RT+2)),
                    # g=1 holds padded rows r0+u+1 (u in [0,RT+1))
                    xt = xpool.tile([128, RT + 2, PW], bf16, tag="xt", name="xt")

                    # g=0: load padded rows [r0, r0+RT+2). Two interleaved-
                    # chunk instructions per sample (4-row runs, 8-row
                    # stride -> ~2KB DMA descriptors) + a 2-row tail.
                    # g=1 = SBUF row-shift copy of g=0 rows [1, RT+2) ->
                    # [0, RT+1): one big-run instruction per sample
                    # (SBUF->SBUF has no small-packet HBM penalty).
                    engs = [nc.sync, nc.gpsimd]
                    c2 = 2 * ldchunk
                    nch = RT // c2  # interleaved chunks per phase
                    for s in range(SB):
                        eng = engs[s % 2]
                        for ph in range(2):
                            eng.dma_start(
                                out=bass.AP(
                                    tensor=xt.tensor,
                                    offset=xt[64 * s : 64 * s + 32].offset
                                    + ph * ldchunk * PW,
                                    ap=[
                                        [XP, 32],
                                        [c2 * PW, nch],
                                        [1, ldchunk * PW],
                                    ],
                                ),
                                in_=dram_ap(
                                    x_d,
                                    s * SPX + (r0 + ph * ldchunk) * PW,
                                    [(PP, 32), (c2 * PW, nch), (1, ldchunk * PW)],
                                ),
                            )
                        eng.dma_start(
                            out=xt[64 * s : 64 * s + 32, RT : RT + 2, :],
                            in_=dram_ap(
                                x_d,
                                s * SPX + (r0 + RT) * PW,
                                [(PP, 32), (1, 2 * PW)],
                            ),
                        )
                        # g=1 row-shift dup on DVE (keeps it off the SDMA
                        # engines, which are the bandwidth bottleneck)
                        nc.vector.tensor_copy(
                            xt[64 * s + 32 : 64 * s + 64, 0 : RT + 1, :],
                            xt[64 * s : 64 * s + 32, 1 : RT + 2, :],
                        )

                    # blocks of 4 output rows (2 stripes x 2 phases), two
                    # samples on disjoint 64x64 PE rectangles
                    for h2 in range(RT // 16):
                        r2 = r0 + 16 * h2
                        ogc = opool.tile(
                            [128, 4, 2, 256], out_dt, tag="ogc", name="ogc"
                        )
                        for b2 in range(4):
                            tlb = 16 * h2 + 4 * b2
                            ps = ppool.tile([128, 512], f32, tag="ps")
                            for k in range(2):
                                row = tlb + 2 * k
                                for kw in range(3):
                                    for s in range(SB):
                                        nc.tensor.matmul(
                                            ps[64 * s : 64 * s + 64, :],
                                            lall2[
                                                64 * s : 64 * s + 64,
                                                3 * k + kw : 3 * k + kw + 1,
                                                :,
                                                :,
                                            ],
                                            bass.AP(
                                                tensor=xt.tensor,
                                                offset=xt[64 * s : 64 * s + 64].offset
                                                + row * PW
                                                + kw,
                                                ap=[[XP, 64], [2 * PW, 2], [1, W]],
                                            ),
                                            start=(k == 0 and kw == 0),
                                            stop=(k == 1 and kw == 2),
                                            skip_group_check=True,
                                        )
                            # PSUM drain + bias on ScalarE (VectorE carries
                            # the x row-shift dup instead)
                            nc.scalar.activation(
                                ogc[:, b2 : b2 + 1, :, :],
                                ps[:],
                                Identity,
                                bias=bvc,
                                scale=1.0,
                            )
                        # stores: phase-plane layout [SB, 2, CO, OH, W]; per
                        # (s,p) group, 8 consecutive plane rows per partition
                        # = 4KB bf16 contiguous runs; issue split over
                        # scalar and sync rings
                        for s in range(SB):
                            for p in range(2):
                                seng = nc.scalar if p == 0 else nc.sync
                                seng.dma_start(
                                    out=dram_ap(
                                        out_d,
                                        (2 * s + p) * OPLANE + (r2 // 2) * W,
                                        [(OH * W, 32), (2 * W, 4), (W, 2), (1, W)],
                                    ),
                                    in_=ogc[
                                        64 * s + 32 * p : 64 * s + 32 * p + 32,
                                        :,
                                        :,
                                        :,
                                    ],
                                )

            if reps_mode == "unroll" and reps_loop > 1:
                for _ in range(reps_loop):
                    conv_body()
            elif reps_loop > 1:
                with tc.For_i(0, reps_loop, 1):
                    conv_body()
            else:
                conv_body()

    nc.finalize()
    return nc


def _get_nc():
    if "nc" not in _CACHE:
        _CACHE["nc"] = _build_nc()
    return _CACHE["nc"]


def kernel(**inputs):
    from concourse.bass_utils import run_bass_kernel_spmd

    nc = _get_nc()
    res = _run(nc, inputs, run_bass_kernel_spmd, trace=False)
    return _gather(res)


def _x_host(x):
    # zero-pad to 258x258 and cast to bf16 on host: device loads become
    # fully contiguous and need no on-chip pad/memset handling
    import ml_dtypes

    xp = np.zeros((x.shape[0], CIN, PH, PW), dtype=ml_dtypes.bfloat16)
    xp[:, :, 1 : H + 1, 1 : W + 1] = np.asarray(x).astype(ml_dtypes.bfloat16)
    return xp


def _run(nc, inputs, run_bass_kernel_spmd, trace=False, trace_kwargs=None):
    in_maps = []
    for c in range(NCORES):
        s = slice(c * SB, (c + 1) * SB)
        in_maps.append(
            {
                "x": _x_host(inputs["x"][s]),
                "condition": np.ascontiguousarray(
                    inputs["condition"][s], dtype=np.float32
                ),
                "layer_pos_embedding": np.ascontiguousarray(
                    inputs["layer_pos_embedding"][s], dtype=np.float32
                ),
                "weights": np.ascontiguousarray(inputs["weights"], dtype=np.float32),
                "bias": np.ascontiguousarray(inputs["bias"], dtype=np.float32),
                "wa_w": np.ascontiguousarray(inputs["wa_w"], dtype=np.float32),
                "wa_b": np.ascontiguousarray(inputs["wa_b"], dtype=np.float32),
                "ba_w": np.ascontiguousarray(inputs["ba_w"], dtype=np.float32),
                "ba_b": np.ascontiguousarray(inputs["ba_b"], dtype=np.float32),
            }
        )
    kwargs = {}
    if trace:
        kwargs["trace"] = True
        if trace_kwargs:
            kwargs["trace_kwargs"] = trace_kwargs
    return run_bass_kernel_spmd(nc, in_maps, core_ids=list(range(NCORES)), **kwargs)


def _gather(res):
    # device output is phase-planes [SB, 2, COUT, OH, W]; de-interleave the
    # two phases back into [SB, COUT, H, W] and cast to f32 on host
    full = np.empty((B, COUT, H, W), dtype=np.float32)
    for c in range(NCORES):
        o = np.asarray(res.results[c]["out"]).astype(np.float32)
        full[c * SB : (c + 1) * SB, :, 0::2, :] = o[:, 0]
        full[c * SB : (c + 1) * SB, :, 1::2, :] = o[:, 1]
    return full


# revision 21
# speedup vs baseline: 3.0202x; 1.1415x over previous
"""BatchAdaptiveConv2d Trainium2 kernel (8 NeuronCores, data parallel).

Math: out[b] = conv2d_same(x[b], W * wadapt[b, ci]) + bias * badapt[b]
 - wadapt[b] = cat(cond[b], lpe[b]) @ wa_w.T + wa_b    (per-sample Cin scale)
 - badapt[b] = cat(cond[b], lpe[b]) @ ba_w.T + ba_b    (per-sample Cout bias scale)

Per-core plan (2 samples per core), m64 high-K matmul formulation:
 - x is zero-padded + bf16-cast on the HOST to [SB, CIN, 258, 258]: device
   loads are fully contiguous (no memsets / edge cases) and issued in
   ~4-row chunks so DMA descriptors stay in the ~2KB sweet spot.
 - x tile [128 = (s, g, ci), RT+1, 258] bf16: g=0 holds x row t-1, g=1
   holds x row t at tile row t. g=0 loaded from HBM; g=1 is an on-chip
   row-shifted SBUF->SBUF copy (chunked) + a 1-row HBM top-up.
 - Each PSUM block covers 4 output rows (2 stripes x 2 phases p); the two
   samples run on disjoint 64x64 PE rectangles so their matmul streams
   overlap: 12 bf16 matmuls K=64=(g,ci), M=64=(p,co), N=512 per block.
 - lhsT slots [64=(g,ci), (p,co)] = W[ci,co,2k+g-p,kw]*wadapt[s,ci] built
   on-chip into a zeroed [128, 6, 2, 32] bf16 table via DVE copies.
 - PSUM -> SBUF via ScalarE Identity-activation with per-partition bias
   (bias*badapt at (s,p?,co)), writing bf16.
 - Output dram layout is PHASE-PLANES [SB, 2, COUT, H/2, W] bf16 so each
   partition's store is 8 consecutive plane rows = 4KB contiguous runs;
   the host de-interleaves phases and casts to f32 in _gather.
"""

import numpy as np

B, CIN, COUT, KK, H, W = 16, 32, 32, 3, 256, 256
EMB = 256
NCORES = 8
SB = B // NCORES  # samples per core
PH, PW = H + 2, W + 2  # host-padded x plane (258 x 258)
OH = H // 2  # phase-plane rows

_CACHE = {}

RT = 32  # output rows per x tile


def _build_nc(reps_loop=1, reps_mode="unroll", out_bf16=True, rt=RT,
              psum_bufs=8, x_bufs=4, og_bufs=4, ldchunk=4, n1024=0):
    import concourse.bacc as bacc
    import concourse.bass as bass
    import concourse.mybir as mybir
    from concourse.tile import TileContext

    f32 = mybir.dt.float32
    bf16 = mybir.dt.bfloat16
    Identity = mybir.ActivationFunctionType.Identity

    nc = bacc.Bacc()

    x_d = nc.declare_dram_parameter("x", [SB, CIN, PH, PW], bf16, isOutput=False)
    cond_d = nc.declare_dram_parameter("condition", [SB, EMB], f32, isOutput=False)
    lpe_d = nc.declare_dram_parameter(
        "layer_pos_embedding", [SB, EMB], f32, isOutput=False
    )
    w_d = nc.declare_dram_parameter("weights", [CIN, COUT, KK, KK], f32, isOutput=False)
    bias_d = nc.declare_dram_parameter("bias", [COUT], f32, isOutput=False)
    waw_d = nc.declare_dram_parameter("wa_w", [CIN, 2 * EMB], f32, isOutput=False)
    wab_d = nc.declare_dram_parameter("wa_b", [CIN], f32, isOutput=False)
    baw_d = nc.declare_dram_parameter("ba_w", [COUT, 2 * EMB], f32, isOutput=False)
    bab_d = nc.declare_dram_parameter("ba_b", [COUT], f32, isOutput=False)
    out_dt = bf16 if out_bf16 else f32
    out_d = nc.declare_dram_parameter(
        "out", [SB, 2, COUT, OH, W], out_dt, isOutput=True
    )

    def dram_ap(handle, offset, dims):
        a = handle[:]
        return bass.AP(tensor=a.tensor, offset=offset, ap=[list(d) for d in dims])

    RT = rt
    NT = H // RT
    HW_ = H * W
    PP = PH * PW  # per-channel padded x plane elems
    SPX = CIN * PP  # per-sample padded x elems
    XP = (RT + 2) * PW  # per-partition x-tile elements
    OPLANE = COUT * OH * W  # one (s, p) output plane

    with TileContext(nc) as tc:
        with (
            tc.tile_pool(name="const", bufs=1) as const,
            tc.tile_pool(name="xin", bufs=x_bufs) as xpool,
            tc.tile_pool(name="ostg", bufs=og_bufs) as opool,
            tc.tile_pool(name="ps", bufs=psum_bufs, space="PSUM") as ppool,
        ):
            # ---- constants replicated x4 across partition quarter groups ----
            wrep = const.tile([128, COUT, KK, KK], f32, tag="wrep")
            nc.sync.dma_start(
                out=wrep.rearrange("q co kh kw -> q (co kh kw)"),
                in_=dram_ap(w_d, 0, [(0, 4), (288, 32), (1, 288)]),
            )
            waw_all = const.tile([128, 2 * EMB], f32, tag="waw_all")
            nc.sync.dma_start(
                out=waw_all, in_=dram_ap(waw_d, 0, [(0, 4), (512, 32), (1, 512)])
            )
            baw_all = const.tile([128, 2 * EMB], f32, tag="baw_all")
            nc.sync.dma_start(
                out=baw_all, in_=dram_ap(baw_d, 0, [(0, 4), (512, 32), (1, 512)])
            )
            wab_all = const.tile([128, 1], f32, tag="wab_all")
            nc.sync.dma_start(
                out=wab_all, in_=dram_ap(wab_d, 0, [(0, 4), (1, 32), (1, 1)])
            )
            bab_all = const.tile([128, 1], f32, tag="bab_all")
            nc.sync.dma_start(
                out=bab_all, in_=dram_ap(bab_d, 0, [(0, 4), (1, 32), (1, 1)])
            )
            bias_all = const.tile([128, 1], f32, tag="bias_all")
            nc.sync.dma_start(
                out=bias_all, in_=dram_ap(bias_d, 0, [(0, 4), (1, 32), (1, 1)])
            )

            # ib: [128=(s,g,ci), 512] = cat(cond[s], lpe[s]) per sample-half
            ib = const.tile([128, 2 * EMB], f32, tag="ib")
            for s in range(SB):
                nc.sync.dma_start(
                    out=ib[64 * s : 64 * s + 64, 0:EMB],
                    in_=dram_ap(cond_d, s * EMB, [(0, 64), (1, EMB)]),
                )
                nc.sync.dma_start(
                    out=ib[64 * s : 64 * s + 64, EMB : 2 * EMB],
                    in_=dram_ap(lpe_d, s * EMB, [(0, 64), (1, EMB)]),
                )

            # wadapt[(s,g,ci), 1] for both samples in one shot
            scr = const.tile([128, 2 * EMB], f32, tag="scr")
            wad = const.tile([128, 1], f32, tag="wad")
            nc.vector.tensor_mul(scr, waw_all, ib)
            nc.vector.reduce_sum(wad, scr, axis=mybir.AxisListType.X)
            nc.vector.tensor_add(wad, wad, wab_all)

            # modulated weights [(s,g,ci), co, kh, kw]
            wmod = const.tile([128, COUT, KK, KK], f32, tag="wmod")
            nc.vector.tensor_scalar_mul(
                wmod.rearrange("q co kh kw -> q (co kh kw)"),
                wrep.rearrange("q co kh kw -> q (co kh kw)"),
                wad,
            )

            # lhsT table: 6 slots (k in {0,1} x 3 kw) of [64=(g,ci), (p2,co)];
            # samples use disjoint 64x64 PE rectangles.
            lall2 = const.tile([128, 6, 2, 32], bf16, tag="lall2")
            nc.vector.memset(lall2.rearrange("q a b c -> q (a b c)"), 0.0)
            for s in range(SB):
                for g in range(2):
                    base = 64 * s + 32 * g
                    for k in range(2):
                        for p in range(2):
                            kh = 2 * k + g - p
                            if not (0 <= kh < KK):
                                continue
                            nc.vector.tensor_copy(
                                lall2[
                                    base : base + 32, 3 * k : 3 * k + 3, p : p + 1, :
                                ],
                                wmod[
                                    base : base + 32, :, kh : kh + 1, :
                                ].transpose([0, 3, 2, 1]),
                            )
            # combined bias vec [(s,g->p,co), 1] = bias[co]*badapt[s,co]
            scr3 = const.tile([128, 2 * EMB], f32, tag="scr3")
            badc = const.tile([128, 1], f32, tag="badc")
            nc.vector.tensor_mul(scr3, baw_all, ib)
            nc.vector.reduce_sum(badc, scr3, axis=mybir.AxisListType.X)
            nc.vector.tensor_add(badc, badc, bab_all)
            bvc = const.tile([128, 1], f32, tag="bvc")
            nc.vector.tensor_mul(bvc, badc, bias_all)

            # ---- main loop over row tiles ----
            def conv_body():
                for t in range(NT):
                    r0 = RT * t
                    # tile rows: g=0 holds padded rows r0+u (u in [0,RT+2)),
                    # g=1 holds padded rows r0+u+1 (u in [0,RT+1))
                    xt = xpool.tile([128, RT + 2, PW], bf16, tag="xt", name="xt")

                    # g=0: load padded rows [r0, r0+RT+2). Two interleaved-
                    # chunk instructions per sample (4-row runs, 8-row
                    # stride -> ~2KB DMA descriptors) + a 2-row tail.
                    # g=1 = SBUF row-shift copy of g=0 rows [1, RT+2) ->
                    # [0, RT+1): one big-run instruction per sample
                    # (SBUF->SBUF has no small-packet HBM penalty).
                    engs = [nc.sync, nc.gpsimd]
                    c2 = 2 * ldchunk
                    nch = RT // c2  # interleaved chunks per phase
                    for s in range(SB):
                        eng = engs[s % 2]
                        for ph in range(2):
                            eng.dma_start(
                                out=bass.AP(
                                    tensor=xt.tensor,
                                    offset=xt[64 * s : 64 * s + 32].offset
                                    + ph * ldchunk * PW,
                                    ap=[
                                        [XP, 32],
                                        [c2 * PW, nch],
                                        [1, ldchunk * PW],
                                    ],
                                ),
                                in_=dram_ap(
                                    x_d,
                                    s * SPX + (r0 + ph * ldchunk) * PW,
                                    [(PP, 32), (c2 * PW, nch), (1, ldchunk * PW)],
                                ),
                            )
                        eng.dma_start(
                            out=xt[64 * s : 64 * s + 32, RT : RT + 2, :],
                            in_=dram_ap(
                                x_d,
                                s * SPX + (r0 + RT) * PW,
                                [(PP, 32), (1, 2 * PW)],
                            ),
                        )
                        # g=1 row-shift dup on DVE (keeps it off the SDMA
                        # engines, which are the bandwidth bottleneck)
                        nc.vector.tensor_copy(
                            xt[64 * s + 32 : 64 * s + 64, 0 : RT + 1, :],
                            xt[64 * s : 64 * s + 32, 1 : RT + 2, :],
                        )

                    # blocks of (2*NS) output rows (NS stripes x 2 phases),
                    # two samples on disjoint 64x64 PE rectangles
                    NS = 4 if n1024 else 2  # N-stripes per matmul
                    NB = 16 // (2 * NS)  # psum blocks per h2 group
                    for h2 in range(RT // 16):
                        r2 = r0 + 16 * h2
                        ogc = opool.tile(
                            [128, NB, NS, 256], out_dt, tag="ogc", name="ogc"
                        )
                        for b2 in range(NB):
                            tlb = 16 * h2 + 2 * NS * b2
                            ps = ppool.tile([128, NS * 256], f32, tag="ps")
                            for k in range(2):
                                row = tlb + 2 * k
                                for kw in range(3):
                                    for s in range(SB):
                                        nc.tensor.matmul(
                                            ps[64 * s : 64 * s + 64, :],
                                            lall2[
                                                64 * s : 64 * s + 64,
                                                3 * k + kw : 3 * k + kw + 1,
                                                :,
                                                :,
                                            ],
                                            bass.AP(
                                                tensor=xt.tensor,
                                                offset=xt[64 * s : 64 * s + 64].offset
                                                + row * PW
                                                + kw,
                                                ap=[[XP, 64], [2 * PW, NS], [1, W]],
                                            ),
                                            start=(k == 0 and kw == 0),
                                            stop=(k == 1 and kw == 2),
                                            skip_group_check=True,
                                        )
                            # PSUM drain + bias on ScalarE (VectorE carries
                            # the x row-shift dup instead)
                            nc.scalar.activation(
                                ogc[:, b2 : b2 + 1, :, :],
                                ps[:],
                                Identity,
                                bias=bvc,
                                scale=1.0,
                            )
                        # stores: phase-plane layout [SB, 2, CO, OH, W]; per
                        # (s,p) group, 8 consecutive plane rows per partition
                        # = 4KB bf16 contiguous runs; issue split over
                        # scalar and sync rings
                        for s in range(SB):
                            for p in range(2):
                                seng = nc.scalar if p == 0 else nc.sync
                                seng.dma_start(
                                    out=dram_ap(
                                        out_d,
                                        (2 * s + p) * OPLANE + (r2 // 2) * W,
                                        [
                                            (OH * W, 32),
                                            (NS * W, NB),
                                            (W, NS),
                                            (1, W),
                                        ],
                                    ),
                                    in_=ogc[
                                        64 * s + 32 * p : 64 * s + 32 * p + 32,
                                        :,
                                        :,
                                        :,
                                    ],
                                )

            if reps_mode == "unroll" and reps_loop > 1:
                for _ in range(reps_loop):
                    conv_body()
            elif reps_loop > 1:
                with tc.For_i(0, reps_loop, 1):
                    conv_body()
            else:
                conv_body()

    nc.finalize()
    return nc


def _get_nc():
    if "nc" not in _CACHE:
        _CACHE["nc"] = _build_nc()
    return _CACHE["nc"]


def kernel(**inputs):
    from concourse.bass_utils import run_bass_kernel_spmd

    nc = _get_nc()
    res = _run(nc, inputs, run_bass_kernel_spmd, trace=False)
    return _gather(res)


def _x_host(x):
    # zero-pad to 258x258 and cast to bf16 on host: device loads become
    # fully contiguous and need no on-chip pad/memset handling
    import ml_dtypes

    xp = np.zeros((x.shape[0], CIN, PH, PW), dtype=ml_dtypes.bfloat16)
    xp[:, :, 1 : H + 1, 1 : W + 1] = np.asarray(x).astype(ml_dtypes.bfloat16)
    return xp


def _run(nc, inputs, run_bass_kernel_spmd, trace=False, trace_kwargs=None):
    in_maps = []
    for c in range(NCORES):
        s = slice(c * SB, (c + 1) * SB)
        in_maps.append(
            {
                "x": _x_host(inputs["x"][s]),
                "condition": np.ascontiguousarray(
                    inputs["condition"][s], dtype=np.float32
                ),
                "layer_pos_embedding": np.ascontiguousarray(
                    inputs["layer_pos_embedding"][s], dtype=np.float32
                ),
                "weights": np.ascontiguousarray(inputs["weights"], dtype=np.float32),
                "bias": np.ascontiguousarray(inputs["bias"], dtype=np.float32),
                "wa_w": np.ascontiguousarray(inputs["wa_w"], dtype=np.float32),
                "wa_b": np.ascontiguousarray(inputs["wa_b"], dtype=np.float32),
                "ba_w": np.ascontiguousarray(inputs["ba_w"], dtype=np.float32),
                "ba_b": np.ascontiguousarray(inputs["ba_b"], dtype=np.float32),
            }
        )
    kwargs = {}
    if trace:
        kwargs["trace"] = True
        if trace_kwargs:
            kwargs["trace_kwargs"] = trace_kwargs
    return run_bass_kernel_spmd(nc, in_maps, core_ids=list(range(NCORES)), **kwargs)


def _gather(res):
    # device output is phase-planes [SB, 2, COUT, OH, W]; de-interleave the
    # two phases back into [SB, COUT, H, W] and cast to f32 on host
    full = np.empty((B, COUT, H, W), dtype=np.float32)
    for c in range(NCORES):
        o = np.asarray(res.results[c]["out"]).astype(np.float32)
        full[c * SB : (c + 1) * SB, :, 0::2, :] = o[:, 0]
        full[c * SB : (c + 1) * SB, :, 1::2, :] = o[:, 1]
    return full
